# revision 11
# baseline (speedup 1.0000x reference)
"""Trainium2 Bass kernel for a 2-layer tanh RNN (B=64, T=512, V=128, H=1024).

Data-parallel over batch across 8 NeuronCores (8 rows/core), weights
replicated and SBUF-resident. Per core (v2, software-wavefront):

  - Layer-0 recurrence with x fused in: h0_t = tanh([x_t, h0_{t-1}] @ [Wx0; Wh0] + b0)
    (V=128 is exactly one K-chunk; x^T is SBUF-resident)
  - U1 = H0 @ Wx1 + b1 computed in 16-step chunks from SBUF-resident h0^T
    blocks, staged through DRAM
  - Layer-1 recurrence: h1_t = tanh(U1_t + h1_{t-1} @ Wh1), lagging layer 0
    by LAG steps; its matmuls fill layer-0's serial-chain latency (and vice
    versa) since the PE executes in order.

Recurrent matmul layout: stationary = h^T chunks [K=128, M=8] (tiny
self-load), moving = Wh chunks [K=128, N=512] streamed at full rate in
float32r. U1 is added via a K=8 identity matmul in the same PSUM
accumulation group. h rows -> h^T via PE transpose each step.
"""

import numpy as np

B, T, V, H = 64, 512, 128, 1024
NCORES = 8
BL = B // NCORES  # 8 batch rows per core
KC = H // 128     # 8 contraction chunks per H
NH = H // 512     # 2 free-dim halves
LAG = 16          # layer-1 lag (steps); also the U1 chunk size
CH = 16           # steps per U1 chunk / h0T block

_cache = {}


def _build(t_dev=T, bias0=False, bias1=False):
    import concourse.bacc as bacc
    import concourse.tile as tile
    import concourse.mybir as mybir

    F32 = mybir.dt.float32
    F32R = mybir.dt.float32r
    TANH = mybir.ActivationFunctionType.Tanh

    assert t_dev % CH == 0
    n_ch = t_dev // CH

    nc = bacc.Bacc("TRN2", target_bir_lowering=False, debug=False,
                   num_devices=NCORES)

    # ---- I/O ----
    xT = nc.dram_tensor("xT", [V, t_dev * BL], F32, kind="ExternalInput").ap()
    hT_init = nc.dram_tensor("hT_init", [2, 128, KC * BL], F32,
                             kind="ExternalInput").ap()
    wx0 = nc.dram_tensor("wx0", [V, H], F32, kind="ExternalInput").ap()
    wh0 = nc.dram_tensor("wh0", [KC, 128, H], F32, kind="ExternalInput").ap()
    wx1 = nc.dram_tensor("wx1", [KC, 128, H], F32, kind="ExternalInput").ap()
    wh1 = nc.dram_tensor("wh1", [KC, 128, H], F32, kind="ExternalInput").ap()
    b0 = nc.dram_tensor("b0", [1, H], F32, kind="ExternalInput").ap()
    b1 = nc.dram_tensor("b1", [1, H], F32, kind="ExternalInput").ap()
    eye8 = nc.dram_tensor("eye8", [8, 8], F32, kind="ExternalInput").ap()
    ones1 = nc.dram_tensor("ones1", [1, 128], F32, kind="ExternalInput").ap()

    outs = nc.dram_tensor("outs", [BL, t_dev, H], F32,
                          kind="ExternalOutput").ap()
    hlast = nc.dram_tensor("hlast", [2, BL, H], F32,
                           kind="ExternalOutput").ap()

    U1 = nc.dram_tensor("U1", [t_dev, BL, H], F32R).ap()

    with tile.TileContext(nc) as tc:
        with tc.tile_pool(name="consts", bufs=1) as consts, \
             tc.tile_pool(name="wpool", bufs=1) as wpool, \
             tc.tile_pool(name="xpool", bufs=1) as xpool, \
             tc.tile_pool(name="u1step", bufs=4) as u1step_pool, \
             tc.tile_pool(name="hrow0", bufs=2) as hrow0_pool, \
             tc.tile_pool(name="hrow1", bufs=2) as hrow1_pool, \
             tc.tile_pool(name="hblk", bufs=3) as hblk_pool, \
             tc.tile_pool(name="h1t", bufs=2) as h1t_pool, \
             tc.tile_pool(name="stage", bufs=2) as stage_pool, \
             tc.tile_pool(name="zb", bufs=2, space="PSUM") as zb_pool, \
             tc.tile_pool(name="zd", bufs=2, space="PSUM") as zd_pool, \
             tc.tile_pool(name="tps", bufs=2, space="PSUM") as tps_pool, \
             tc.tile_pool(name="aps", bufs=2, space="PSUM") as aps_pool:

            # ---- constants / weights ----
            eye_r = consts.tile([8, 8], F32R)
            nc.sync.dma_start(eye_r[:], eye8.bitcast(F32R))
            eye_f = consts.tile([8, 8], F32)
            nc.sync.dma_start(eye_f[:], eye8)
            ones_r = consts.tile([1, 128], F32R)
            nc.sync.dma_start(ones_r[:], ones1.bitcast(F32R))
            b0_t = consts.tile([1, H], F32R)
            nc.sync.dma_start(b0_t[:], b0.bitcast(F32R))
            b1_t = consts.tile([1, H], F32R)
            nc.sync.dma_start(b1_t[:], b1.bitcast(F32R))
            hT0_init = consts.tile([128, KC * BL], F32R)
            nc.sync.dma_start(hT0_init[:], hT_init[0].bitcast(F32R))
            hT1_init = consts.tile([128, KC * BL], F32R)
            nc.sync.dma_start(hT1_init[:], hT_init[1].bitcast(F32R))

            wx0_t = wpool.tile([V, H], F32R)
            nc.sync.dma_start(wx0_t[:], wx0.bitcast(F32R))
            w_tiles = {}
            for name, src in (("wh0", wh0), ("wx1", wx1), ("wh1", wh1)):
                wt = wpool.tile([128, KC * H], F32R, tag=name)
                nc.sync.dma_start(
                    wt[:].rearrange("p (k n) -> p k n", k=KC, n=H),
                    src.bitcast(F32R).rearrange("k p n -> p k n"))
                w_tiles[name] = wt[:].rearrange("p (k n) -> p k n", k=KC, n=H)
            wh0_r, wx1_r, wh1_r = (w_tiles["wh0"], w_tiles["wx1"],
                                   w_tiles["wh1"])

            # x^T resident, columns (t, b) t-major
            xT_t = xpool.tile([V, t_dev * BL], F32R)
            nc.sync.dma_start(xT_t[:], xT.bitcast(F32R))

            hblk_tiles = []   # rolling h0^T block tiles, one per CH steps
            h1t_tiles = []    # rolling h1^T tiles, one per step

            # ---------- emission helpers ----------
            def b_stat(t, k):
                """stationary h0^T chunk k for layer-0 step t."""
                if t == 0:
                    return hT0_init[:].rearrange(
                        "p (k b) -> p k b", k=KC, b=BL)[:, k, :]
                blk = hblk_tiles[(t - 1) // CH][:].rearrange(
                    "p (k s b) -> p k s b", k=KC, s=CH, b=BL)
                return blk[:, k, (t - 1) % CH, :]

            def emit_b_mms(t, nh):
                """layer-0 step t, half nh: z half = [x_t, h_{t-1}] @ W + b0."""
                if t % CH == 0 and t // CH == len(hblk_tiles):
                    hblk_t = hblk_pool.tile([128, KC * CH * BL], F32R,
                                            tag="hblk")
                    hblk_tiles.append(hblk_t)
                zt = zb_pool.tile([BL, 512], F32, tag="zb")
                psn = zt[:]
                nc.tensor.matmul(psn, xT_t[:, t * BL:(t + 1) * BL],
                                 wx0_t[:, nh * 512:(nh + 1) * 512],
                                 start=True, stop=False)
                if bias0:
                    nc.tensor.matmul(psn, ones_r[:],
                                     b0_t[:, nh * 512:(nh + 1) * 512],
                                     start=False, stop=False)
                for k in range(KC):
                    nc.tensor.matmul(psn, b_stat(t, k),
                                     wh0_r[:, k, nh * 512:(nh + 1) * 512],
                                     start=False, stop=(k == KC - 1))
                if nh == 0:
                    h_rows = hrow0_pool.tile([BL, H], F32, tag="hrow0")
                    emit_b_mms.h_rows = h_rows
                nc.scalar.activation(
                    emit_b_mms.h_rows[:, nh * 512:(nh + 1) * 512], psn, TANH)

            def emit_b_tail(t):
                """layer-0 step t: transpose h rows into the h0^T block."""
                h_rows = emit_b_mms.h_rows
                tp = tps_pool.tile([128, KC * BL], F32, tag="tps")
                for k in range(KC):
                    nc.tensor.transpose(tp[:, k * BL:(k + 1) * BL],
                                        h_rows[:, k * 128:(k + 1) * 128],
                                        eye_f[:])
                blk = hblk_tiles[t // CH][:].rearrange(
                    "p (k s b) -> p k s b", k=KC, s=CH, b=BL)
                nc.vector.tensor_copy(
                    blk[:, :, t % CH, :],
                    tp[:].rearrange("p (k b) -> p k b", k=KC, b=BL))
                if t == t_dev - 1:
                    nc.sync.dma_start(hlast[0], h_rows[:])

            def emit_c_chunk(m):
                """U1 chunk m: rows (t, b) for t in [m*CH, (m+1)*CH)."""
                blk = hblk_tiles[m][:].rearrange(
                    "p (k s b) -> p k s b", k=KC, s=CH, b=BL)
                u_sb = stage_pool.tile([128, H], F32R, tag="stage")
                for nh in range(NH):
                    psn = aps_pool.tile([128, 512], F32, tag="aps")
                    first = True
                    if bias1:
                        nc.tensor.matmul(psn[:], ones_r[:],
                                         b1_t[:, nh * 512:(nh + 1) * 512],
                                         start=True, stop=False)
                        first = False
                    for k in range(KC):
                        nc.tensor.matmul(
                            psn[:],
                            blk[:, k, :, :],
                            wx1_r[:, k, nh * 512:(nh + 1) * 512],
                            start=first, stop=(k == KC - 1))
                        first = False
                    nc.vector.tensor_copy(
                        u_sb[:, nh * 512:(nh + 1) * 512], psn[:])
                nc.sync.dma_start(U1[m * CH:(m + 1) * CH, :, :], u_sb[:])

            def d_stat(t, k):
                if t == 0:
                    return hT1_init[:].rearrange(
                        "p (k b) -> p k b", k=KC, b=BL)[:, k, :]
                return h1t_tiles[t - 1][:].rearrange(
                    "p (k b) -> p k b", k=KC, b=BL)[:, k, :]

            def emit_d_mms(t, nh):
                """layer-1 step t, half nh."""
                if nh == 0:
                    u_t = u1step_pool.tile([BL, H], F32R, tag="u1")
                    nc.sync.dma_start(u_t[:], U1[t, :, :])
                    emit_d_mms.u_t = u_t
                u_t = emit_d_mms.u_t
                zt = zd_pool.tile([BL, 512], F32, tag="zd")
                psn = zt[:]
                nc.tensor.matmul(psn, eye_r[:],
                                 u_t[:, nh * 512:(nh + 1) * 512],
                                 start=True, stop=False)
                for k in range(KC):
                    nc.tensor.matmul(psn, d_stat(t, k),
                                     wh1_r[:, k, nh * 512:(nh + 1) * 512],
                                     start=False, stop=(k == KC - 1))
                if nh == 0:
                    emit_d_mms.h_rows = hrow1_pool.tile([BL, H], F32,
                                                        tag="hrow1")
                nc.scalar.activation(
                    emit_d_mms.h_rows[:, nh * 512:(nh + 1) * 512], psn, TANH)

            def emit_d_tail(t):
                h_rows = emit_d_mms.h_rows
                nc.sync.dma_start(outs[:, t, :], h_rows[:])
                if t == t_dev - 1:
                    nc.sync.dma_start(hlast[1], h_rows[:])
                    return
                tp = tps_pool.tile([128, KC * BL], F32, tag="tps")
                for k in range(KC):
                    nc.tensor.transpose(tp[:, k * BL:(k + 1) * BL],
                                        h_rows[:, k * 128:(k + 1) * 128],
                                        eye_f[:])
                h1t = h1t_pool.tile([128, KC * BL], F32R, tag="h1t")
                nc.vector.tensor_copy(
                    h1t[:].rearrange("p (k b) -> p k b", k=KC, b=BL),
                    tp[:].rearrange("p (k b) -> p k b", k=KC, b=BL))
                h1t_tiles.append(h1t)

            # ---------- wavefront ----------
            # PE order per tick t:
            #   B:MMs(t) half0 | D:T(t-LAG-1) | B:MMs(t) half1 | D:MMs(t-LAG)
            #   | B:T(t) | [C chunk every CH ticks]
            for t in range(t_dev + LAG):
                td = t - LAG
                if t < t_dev:
                    emit_b_mms(t, 0)
                if 0 < td < t_dev:
                    emit_d_tail(td - 1)
                if t < t_dev:
                    emit_b_mms(t, 1)
                if 0 <= td < t_dev:
                    emit_d_mms(td, 0)
                    emit_d_mms(td, 1)
                if t < t_dev:
                    emit_b_tail(t)
                    if t % CH == CH - 1:
                        emit_c_chunk(t // CH)
                if td == t_dev - 1:
                    emit_d_tail(td)

    nc.compile()
    return nc


def _prep_inputs(x, h, c, Wx0, Wh0, b0, Wx1, Wh1, b1, t_dev=T):
    x = np.asarray(x, dtype=np.float32)
    h = np.asarray(h, dtype=np.float32)
    shared = {
        "wx0": np.ascontiguousarray(np.asarray(Wx0, np.float32)),
        "wh0": np.ascontiguousarray(
            np.asarray(Wh0, np.float32).reshape(KC, 128, H)),
        "wx1": np.ascontiguousarray(
            np.asarray(Wx1, np.float32).reshape(KC, 128, H)),
        "wh1": np.ascontiguousarray(
            np.asarray(Wh1, np.float32).reshape(KC, 128, H)),
        "b0": np.asarray(b0, np.float32).reshape(1, H),
        "b1": np.asarray(b1, np.float32).reshape(1, H),
        "eye8": np.eye(8, dtype=np.float32),
        "ones1": np.ones((1, 128), dtype=np.float32),
    }
    in_maps = []
    for ci in range(NCORES):
        bs = slice(ci * BL, (ci + 1) * BL)
        x_loc = x[bs, :t_dev]                             # (BL, t, V)
        xT_loc = np.ascontiguousarray(
            x_loc.transpose(2, 1, 0)).reshape(V, t_dev * BL)  # (V, (t b))
        hT = np.empty((2, 128, KC * BL), np.float32)
        for l in range(2):
            hl = h[l, bs].T.reshape(KC, 128, BL)          # (hc, p, b)
            hT[l] = np.ascontiguousarray(
                hl.transpose(1, 0, 2)).reshape(128, KC * BL)
        m = dict(shared)
        m["xT"] = xT_loc
        m["hT_init"] = hT
        in_maps.append(m)
    return in_maps


def _bias_flags(b0, b1):
    return bool(np.any(np.asarray(b0))), bool(np.any(np.asarray(b1)))


def _run(in_maps, t_dev=T, bias0=False, bias1=False, trace=False, **kw):
    from concourse import bass_utils
    key = (t_dev, bias0, bias1)
    if key not in _cache:
        _cache[key] = _build(t_dev, bias0, bias1)
    nc = _cache[key]
    return bass_utils.run_bass_kernel_spmd(
        nc, in_maps, core_ids=list(range(NCORES)), trace=trace, **kw)


def kernel(x, h, c, Wx0, Wh0, b0, Wx1, Wh1, b1):
    bias0, bias1 = _bias_flags(b0, b1)
    in_maps = _prep_inputs(x, h, c, Wx0, Wh0, b0, Wx1, Wh1, b1)
    res = _run(in_maps, bias0=bias0, bias1=bias1)
    outs = np.concatenate([res.results[ci]["outs"] for ci in range(NCORES)],
                          axis=0)
    h_last = np.concatenate([res.results[ci]["hlast"] for ci in range(NCORES)],
                            axis=1)
    return outs, np.ascontiguousarray(h_last), np.asarray(c, np.float32)


# revision 30
# speedup vs baseline: 1.2092x; 1.2092x over previous
"""Trainium2 Bass kernel for a 2-layer tanh RNN (B=64, T=512, V=128, H=1024).

Data-parallel over batch across 8 NeuronCores (8 rows/core), weights
replicated and SBUF-resident. Per core (v2, software-wavefront):

  - Layer-0 recurrence with x fused in: h0_t = tanh([x_t, h0_{t-1}] @ [Wx0; Wh0] + b0)
    (V=128 is exactly one K-chunk; x^T is SBUF-resident)
  - U1 = H0 @ Wx1 + b1 computed in 16-step chunks from SBUF-resident h0^T
    blocks, staged through DRAM
  - Layer-1 recurrence: h1_t = tanh(U1_t + h1_{t-1} @ Wh1), lagging layer 0
    by LAG steps; its matmuls fill layer-0's serial-chain latency (and vice
    versa) since the PE executes in order.

Recurrent matmul layout: stationary = h^T chunks [K=128, M=8] (tiny
self-load), moving = Wh chunks [K=128, N=512] streamed at full rate in
float32r. U1 is added via a K=8 identity matmul in the same PSUM
accumulation group. h rows -> h^T via PE transpose each step.
"""

import numpy as np

B, T, V, H = 64, 512, 128, 1024
NCORES = 8
BL = B // NCORES  # 8 batch rows per core
KC = H // 128     # 8 contraction chunks per H
NH = H // 512     # 2 free-dim halves
LAG = 16          # layer-1 lag (steps); also the U1 chunk size
CH = 16           # steps per U1 chunk / h0T block

_cache = {}


def _build(t_dev=T, bias0=False, bias1=False, reps=1):
    import concourse.bacc as bacc
    import concourse.tile as tile
    import concourse.mybir as mybir

    F32 = mybir.dt.float32
    F32R = mybir.dt.float32r
    TANH = mybir.ActivationFunctionType.Tanh

    assert t_dev % CH == 0
    n_ch = t_dev // CH

    nc = bacc.Bacc("TRN2", target_bir_lowering=False, debug=False,
                   num_devices=NCORES)

    # ---- I/O ----
    xT = nc.dram_tensor("xT", [V, t_dev * BL], F32, kind="ExternalInput").ap()
    hT_init = nc.dram_tensor("hT_init", [2, 128, KC * BL], F32,
                             kind="ExternalInput").ap()
    wx0 = nc.dram_tensor("wx0", [V, H], F32, kind="ExternalInput").ap()
    wh0 = nc.dram_tensor("wh0", [KC, 128, H], F32, kind="ExternalInput").ap()
    wx1 = nc.dram_tensor("wx1", [KC, 128, H], F32, kind="ExternalInput").ap()
    wh1 = nc.dram_tensor("wh1", [KC, 128, H], F32, kind="ExternalInput").ap()
    b0 = nc.dram_tensor("b0", [1, H], F32, kind="ExternalInput").ap()
    b1 = nc.dram_tensor("b1", [1, H], F32, kind="ExternalInput").ap()
    eye8 = nc.dram_tensor("eye8", [8, 8], F32, kind="ExternalInput").ap()
    ones1 = nc.dram_tensor("ones1", [1, 128], F32, kind="ExternalInput").ap()

    outs = nc.dram_tensor("outs", [BL, t_dev, H], F32,
                          kind="ExternalOutput").ap()
    hlast = nc.dram_tensor("hlast", [2, BL, H], F32,
                           kind="ExternalOutput").ap()

    U1 = nc.dram_tensor("U1", [t_dev, BL, H], F32R).ap()

    with tile.TileContext(nc) as tc:
        with tc.tile_pool(name="consts", bufs=1) as consts, \
             tc.tile_pool(name="wpool", bufs=1) as wpool, \
             tc.tile_pool(name="xpool", bufs=1) as xpool, \
             tc.tile_pool(name="u1step", bufs=4) as u1step_pool, \
             tc.tile_pool(name="hrow0", bufs=2) as hrow0_pool, \
             tc.tile_pool(name="hrow1", bufs=2) as hrow1_pool, \
             tc.tile_pool(name="hblk", bufs=3) as hblk_pool, \
             tc.tile_pool(name="h1t", bufs=2) as h1t_pool, \
             tc.tile_pool(name="stage", bufs=2) as stage_pool, \
             tc.tile_pool(name="zb", bufs=2, space="PSUM") as zb_pool, \
             tc.tile_pool(name="zd", bufs=2, space="PSUM") as zd_pool, \
             tc.tile_pool(name="tps", bufs=2, space="PSUM") as tps_pool, \
             tc.tile_pool(name="aps", bufs=2, space="PSUM") as aps_pool:

            # ---- constants / weights ----
            eye_r = consts.tile([8, 8], F32R)
            nc.sync.dma_start(eye_r[:], eye8.bitcast(F32R))
            eye_f = consts.tile([8, 8], F32)
            nc.sync.dma_start(eye_f[:], eye8)
            ones_r = consts.tile([1, 128], F32R)
            nc.sync.dma_start(ones_r[:], ones1.bitcast(F32R))
            b0_t = consts.tile([1, H], F32R)
            nc.sync.dma_start(b0_t[:], b0.bitcast(F32R))
            b1_t = consts.tile([1, H], F32R)
            nc.sync.dma_start(b1_t[:], b1.bitcast(F32R))
            hT0_init = consts.tile([128, KC * BL], F32R)
            nc.sync.dma_start(hT0_init[:], hT_init[0].bitcast(F32R))
            hT1_init = consts.tile([128, KC * BL], F32R)
            nc.sync.dma_start(hT1_init[:], hT_init[1].bitcast(F32R))

            wx0_t = wpool.tile([V, H], F32R)
            nc.sync.dma_start(wx0_t[:], wx0.bitcast(F32R))
            w_tiles = {}
            for name, src in (("wh0", wh0), ("wx1", wx1), ("wh1", wh1)):
                wt = wpool.tile([128, KC * H], F32R, tag=name)
                wt_r = wt[:].rearrange("p (k n) -> p k n", k=KC, n=H)
                # per-chunk DMAs so the first matmuls don't wait on 4 MB
                for k in range(KC):
                    nc.sync.dma_start(wt_r[:, k, :], src[k].bitcast(F32R))
                w_tiles[name] = wt_r
            wh0_r, wx1_r, wh1_r = (w_tiles["wh0"], w_tiles["wx1"],
                                   w_tiles["wh1"])

            # x^T resident, columns (t, b) t-major; chunked DMA
            xT_t = xpool.tile([V, t_dev * BL], F32R)
            for xc in range(4):
                w = t_dev * BL // 4
                nc.sync.dma_start(xT_t[:, xc * w:(xc + 1) * w],
                                  xT[:, xc * w:(xc + 1) * w].bitcast(F32R))

            hblk_tiles = []   # rolling h0^T block tiles, one per CH steps
            h1t_tiles = []    # rolling h1^T tiles, one per step
            rep_state = {"rep": 0}

            # ---------- emission helpers ----------
            def b_stat(t, k):
                """stationary h0^T chunk k for layer-0 step t."""
                if t == 0:
                    return hT0_init[:].rearrange(
                        "p (k b) -> p k b", k=KC, b=BL)[:, k, :]
                blk = hblk_tiles[(t - 1) // CH][:].rearrange(
                    "p (k s b) -> p k s b", k=KC, s=CH, b=BL)
                return blk[:, k, (t - 1) % CH, :]

            def emit_b_mms(t, nh):
                """layer-0 step t, half nh: z = [x_t, h_{t-1}] @ [Wx0; Wh0]."""
                if t % CH == 0 and t // CH == len(hblk_tiles):
                    hblk_t = hblk_pool.tile([128, KC * CH * BL], F32R,
                                            tag="hblk")
                    hblk_tiles.append(hblk_t)
                zt = zb_pool.tile([BL, 512], F32, tag="zb")
                psn = zt[:]
                nc.tensor.matmul(psn, xT_t[:, t * BL:(t + 1) * BL],
                                 wx0_t[:, nh * 512:(nh + 1) * 512],
                                 start=True, stop=False)
                if bias0:
                    nc.tensor.matmul(psn, ones_r[:],
                                     b0_t[:, nh * 512:(nh + 1) * 512],
                                     start=False, stop=False)
                for k in range(KC):
                    nc.tensor.matmul(psn, b_stat(t, k),
                                     wh0_r[:, k, nh * 512:(nh + 1) * 512],
                                     start=False, stop=(k == KC - 1))
                if nh == 0:
                    h_rows = hrow0_pool.tile([BL, H], F32, tag="hrow0")
                    emit_b_mms.h_rows = h_rows
                nc.scalar.activation(
                    emit_b_mms.h_rows[:, nh * 512:(nh + 1) * 512], psn, TANH)

            def emit_b_tail(t):
                """layer-0 step t: transpose h rows into the h0^T block."""
                h_rows = emit_b_mms.h_rows
                tp = tps_pool.tile([128, KC * BL], F32, tag="tps")
                for k in range(KC):
                    nc.tensor.transpose(tp[:, k * BL:(k + 1) * BL],
                                        h_rows[:, k * 128:(k + 1) * 128],
                                        eye_f[:])
                blk = hblk_tiles[t // CH][:].rearrange(
                    "p (k s b) -> p k s b", k=KC, s=CH, b=BL)
                nc.scalar.copy(
                    blk[:, :, t % CH, :],
                    tp[:].rearrange("p (k b) -> p k b", k=KC, b=BL))
                if t == t_dev - 1:
                    nc.sync.dma_start(hlast[0], h_rows[:])

            def emit_c_chunk(m):
                """U1 chunk m: rows (t, b) for t in [m*CH, (m+1)*CH)."""
                blk = hblk_tiles[m][:].rearrange(
                    "p (k s b) -> p k s b", k=KC, s=CH, b=BL)
                u_sb = stage_pool.tile([128, H], F32R, tag="stage")
                for nh in range(NH):
                    psn = aps_pool.tile([128, 512], F32, tag="aps")
                    first = True
                    if bias1:
                        nc.tensor.matmul(psn[:], ones_r[:],
                                         b1_t[:, nh * 512:(nh + 1) * 512],
                                         start=True, stop=False)
                        first = False
                    for k in range(KC):
                        nc.tensor.matmul(
                            psn[:],
                            blk[:, k, :, :],
                            wx1_r[:, k, nh * 512:(nh + 1) * 512],
                            start=first, stop=(k == KC - 1))
                        first = False
                    nc.vector.tensor_copy(
                        u_sb[:, nh * 512:(nh + 1) * 512], psn[:])
                nc.sync.dma_start(U1[m * CH:(m + 1) * CH, :, :], u_sb[:])

            def d_stat(t, k):
                if t == 0:
                    return hT1_init[:].rearrange(
                        "p (k b) -> p k b", k=KC, b=BL)[:, k, :]
                return h1t_tiles[t - 1][:].rearrange(
                    "p (k b) -> p k b", k=KC, b=BL)[:, k, :]

            def emit_d_mms(t, nh):
                """layer-1 step t, half nh."""
                if nh == 0:
                    u_t = u1step_pool.tile([BL, H], F32R, tag="u1")
                    nc.sync.dma_start(u_t[:], U1[t, :, :])
                    emit_d_mms.u_t = u_t
                u_t = emit_d_mms.u_t
                zt = zd_pool.tile([BL, 512], F32, tag="zd")
                psn = zt[:]
                for k in range(KC):
                    nc.tensor.matmul(psn, d_stat(t, k),
                                     wh1_r[:, k, nh * 512:(nh + 1) * 512],
                                     start=(k == 0), stop=(k == KC - 1))
                nc.vector.tensor_add(psn, psn,
                                     u_t[:, nh * 512:(nh + 1) * 512])
                if nh == 0:
                    emit_d_mms.h_rows = hrow1_pool.tile([BL, H], F32,
                                                        tag="hrow1")
                nc.scalar.activation(
                    emit_d_mms.h_rows[:, nh * 512:(nh + 1) * 512], psn, TANH)

            def emit_d_tail(t):
                h_rows = emit_d_mms.h_rows
                nc.sync.dma_start(outs[:, t, :], h_rows[:])
                if t == t_dev - 1:
                    nc.sync.dma_start(hlast[1], h_rows[:])
                    return
                tp = tps_pool.tile([128, KC * BL], F32, tag="tps")
                for k in range(KC):
                    nc.tensor.transpose(tp[:, k * BL:(k + 1) * BL],
                                        h_rows[:, k * 128:(k + 1) * 128],
                                        eye_f[:])
                h1t = h1t_pool.tile([128, KC * BL], F32R, tag="h1t")
                nc.scalar.copy(
                    h1t[:].rearrange("p (k b) -> p k b", k=KC, b=BL),
                    tp[:].rearrange("p (k b) -> p k b", k=KC, b=BL))
                h1t_tiles.append(h1t)

            # ---------- wavefront ----------
            # PE order per tick t:
            #   B:MMs(t,0) | D:T(td-1) | B:MMs(t,1) | D:MMs(td,0) | B:T(t)
            #   | D:MMs(td,1) | [C chunk every CH ticks]
            for _rep in range(reps):
                hblk_tiles.clear()
                h1t_tiles.clear()
                for t in range(t_dev + LAG):
                    td = t - LAG
                    if t < t_dev:
                        emit_b_mms(t, 0)
                    if 0 < td < t_dev:
                        emit_d_tail(td - 1)
                    if t < t_dev:
                        emit_b_mms(t, 1)
                    if 0 <= td < t_dev:
                        emit_d_mms(td, 0)
                    if t < t_dev:
                        emit_b_tail(t)
                    if 0 <= td < t_dev:
                        emit_d_mms(td, 1)
                    if t < t_dev and t % CH == CH - 1:
                        emit_c_chunk(t // CH)
                    if td == t_dev - 1:
                        emit_d_tail(td)

    nc.compile()
    return nc


def _prep_inputs(x, h, c, Wx0, Wh0, b0, Wx1, Wh1, b1, t_dev=T):
    x = np.asarray(x, dtype=np.float32)
    h = np.asarray(h, dtype=np.float32)
    shared = {
        "wx0": np.ascontiguousarray(np.asarray(Wx0, np.float32)),
        "wh0": np.ascontiguousarray(
            np.asarray(Wh0, np.float32).reshape(KC, 128, H)),
        "wx1": np.ascontiguousarray(
            np.asarray(Wx1, np.float32).reshape(KC, 128, H)),
        "wh1": np.ascontiguousarray(
            np.asarray(Wh1, np.float32).reshape(KC, 128, H)),
        "b0": np.asarray(b0, np.float32).reshape(1, H),
        "b1": np.asarray(b1, np.float32).reshape(1, H),
        "eye8": np.eye(8, dtype=np.float32),
        "ones1": np.ones((1, 128), dtype=np.float32),
    }
    in_maps = []
    for ci in range(NCORES):
        bs = slice(ci * BL, (ci + 1) * BL)
        x_loc = x[bs, :t_dev]                             # (BL, t, V)
        xT_loc = np.ascontiguousarray(
            x_loc.transpose(2, 1, 0)).reshape(V, t_dev * BL)  # (V, (t b))
        hT = np.empty((2, 128, KC * BL), np.float32)
        for l in range(2):
            hl = h[l, bs].T.reshape(KC, 128, BL)          # (hc, p, b)
            hT[l] = np.ascontiguousarray(
                hl.transpose(1, 0, 2)).reshape(128, KC * BL)
        m = dict(shared)
        m["xT"] = xT_loc
        m["hT_init"] = hT
        in_maps.append(m)
    return in_maps


def _bias_flags(b0, b1):
    return bool(np.any(np.asarray(b0))), bool(np.any(np.asarray(b1)))


def _run(in_maps, t_dev=T, bias0=False, bias1=False, reps=1, trace=False,
         **kw):
    from concourse import bass_utils
    key = (t_dev, bias0, bias1, reps)
    if key not in _cache:
        _cache[key] = _build(t_dev, bias0, bias1, reps)
    nc = _cache[key]
    return bass_utils.run_bass_kernel_spmd(
        nc, in_maps, core_ids=list(range(NCORES)), trace=trace, **kw)


def kernel(x, h, c, Wx0, Wh0, b0, Wx1, Wh1, b1):
    bias0, bias1 = _bias_flags(b0, b1)
    in_maps = _prep_inputs(x, h, c, Wx0, Wh0, b0, Wx1, Wh1, b1)
    res = _run(in_maps, bias0=bias0, bias1=bias1)
    outs = np.concatenate([res.results[ci]["outs"] for ci in range(NCORES)],
                          axis=0)
    h_last = np.concatenate([res.results[ci]["hlast"] for ci in range(NCORES)],
                            axis=1)
    return outs, np.ascontiguousarray(h_last), np.asarray(c, np.float32)


# revision 31
# speedup vs baseline: 22.7467x; 18.8117x over previous
"""Trainium2 Bass kernel for a 2-layer tanh RNN (B=64, T=512, V=128, H=1024).

Data-parallel over batch across 8 NeuronCores (8 rows/core, zero
collectives), weights replicated and SBUF-resident. Per core, a
software-wavefront interleaves three streams so the in-order PE never waits
on the serial tanh -> transpose -> copy chain of either recurrence:

  - Layer 0, step t: h0_t = tanh([x_t, h0_{t-1}] @ [Wx0; Wh0] (+ b0))
    (V=128 is exactly one extra contraction chunk; x^T is SBUF-resident, so
    the input projection rides the same PSUM accumulation group for free)
  - U1 = H0 @ Wx1 (+ b1), computed in 16-step chunks (M=128 output rows,
    16x the per-step efficiency) from the SBUF-resident h0^T blocks the
    layer-0 stream produces; staged to DRAM and read back per step
  - Layer 1, step t-LAG: h1 = tanh(U1_t + h1_{t-1} @ Wh1); U1 is injected
    with a DVE add into PSUM (off the PE), tanh on ACT, h rows -> h^T via
    8 PE transposes + an ACT copy (ACT so the DVE adds never delay it).

Recurrent matmul layout: stationary = h^T chunks [K=128, M=8] (tiny
self-load), moving = Wh chunks [K=128, N=512] streamed at 1 column/cycle in
float32r (full rate; fp32 would be 4x slower, and float32r keeps ~1.5e-4
relative accuracy, which the contractive tanh dynamics do not amplify).
Per-tick PE order: B:MMs(t,h0) | D:T(td-1) | B:MMs(t,h1) | D:MMs(td,h0) |
B:T(t) | D:MMs(td,h1) | [C chunk every 16 ticks] — each cross-engine
dependency is covered by ~1.7us of the other stream's matmuls.

Cost-model timeline: ~4.05 ms/core, PE 96% busy; the irreducible floor is
streaming Wh0+Wh1 (2 x 4 MB) through the PE every step at 128 lanes/cycle
= 3.41 us/step-pair (batch-per-core of 8 leaves M=8 of 128, but moving-
operand bandwidth, not M, is the binding constraint at any batch split).
"""

import numpy as np

B, T, V, H = 64, 512, 128, 1024
NCORES = 8
BL = B // NCORES  # 8 batch rows per core
KC = H // 128     # 8 contraction chunks per H
NH = H // 512     # 2 free-dim halves
LAG = 16          # layer-1 lag (steps); also the U1 chunk size
CH = 16           # steps per U1 chunk / h0T block

_cache = {}


def _build(t_dev=T, bias0=False, bias1=False, reps=1):
    import concourse.bacc as bacc
    import concourse.tile as tile
    import concourse.mybir as mybir

    F32 = mybir.dt.float32
    F32R = mybir.dt.float32r
    TANH = mybir.ActivationFunctionType.Tanh

    assert t_dev % CH == 0
    n_ch = t_dev // CH

    nc = bacc.Bacc("TRN2", target_bir_lowering=False, debug=False,
                   num_devices=NCORES)

    # ---- I/O ----
    xT = nc.dram_tensor("xT", [V, t_dev * BL], F32, kind="ExternalInput").ap()
    hT_init = nc.dram_tensor("hT_init", [2, 128, KC * BL], F32,
                             kind="ExternalInput").ap()
    wx0 = nc.dram_tensor("wx0", [V, H], F32, kind="ExternalInput").ap()
    wh0 = nc.dram_tensor("wh0", [KC, 128, H], F32, kind="ExternalInput").ap()
    wx1 = nc.dram_tensor("wx1", [KC, 128, H], F32, kind="ExternalInput").ap()
    wh1 = nc.dram_tensor("wh1", [KC, 128, H], F32, kind="ExternalInput").ap()
    b0 = nc.dram_tensor("b0", [1, H], F32, kind="ExternalInput").ap()
    b1 = nc.dram_tensor("b1", [1, H], F32, kind="ExternalInput").ap()
    eye8 = nc.dram_tensor("eye8", [8, 8], F32, kind="ExternalInput").ap()
    ones1 = nc.dram_tensor("ones1", [1, 128], F32, kind="ExternalInput").ap()

    outs = nc.dram_tensor("outs", [BL, t_dev, H], F32,
                          kind="ExternalOutput").ap()
    hlast = nc.dram_tensor("hlast", [2, BL, H], F32,
                           kind="ExternalOutput").ap()

    U1 = nc.dram_tensor("U1", [t_dev, BL, H], F32R).ap()

    with tile.TileContext(nc) as tc:
        with tc.tile_pool(name="consts", bufs=1) as consts, \
             tc.tile_pool(name="wpool", bufs=1) as wpool, \
             tc.tile_pool(name="xpool", bufs=1) as xpool, \
             tc.tile_pool(name="u1step", bufs=4) as u1step_pool, \
             tc.tile_pool(name="hrow0", bufs=2) as hrow0_pool, \
             tc.tile_pool(name="hrow1", bufs=2) as hrow1_pool, \
             tc.tile_pool(name="hblk", bufs=3) as hblk_pool, \
             tc.tile_pool(name="h1t", bufs=2) as h1t_pool, \
             tc.tile_pool(name="stage", bufs=2) as stage_pool, \
             tc.tile_pool(name="zb", bufs=2, space="PSUM") as zb_pool, \
             tc.tile_pool(name="zd", bufs=2, space="PSUM") as zd_pool, \
             tc.tile_pool(name="tps", bufs=2, space="PSUM") as tps_pool, \
             tc.tile_pool(name="aps", bufs=2, space="PSUM") as aps_pool:

            # ---- constants / weights ----
            eye_r = consts.tile([8, 8], F32R)
            nc.sync.dma_start(eye_r[:], eye8.bitcast(F32R))
            eye_f = consts.tile([8, 8], F32)
            nc.sync.dma_start(eye_f[:], eye8)
            ones_r = consts.tile([1, 128], F32R)
            nc.sync.dma_start(ones_r[:], ones1.bitcast(F32R))
            b0_t = consts.tile([1, H], F32R)
            nc.sync.dma_start(b0_t[:], b0.bitcast(F32R))
            b1_t = consts.tile([1, H], F32R)
            nc.sync.dma_start(b1_t[:], b1.bitcast(F32R))
            hT0_init = consts.tile([128, KC * BL], F32R)
            nc.sync.dma_start(hT0_init[:], hT_init[0].bitcast(F32R))
            hT1_init = consts.tile([128, KC * BL], F32R)
            nc.sync.dma_start(hT1_init[:], hT_init[1].bitcast(F32R))

            wx0_t = wpool.tile([V, H], F32R)
            nc.sync.dma_start(wx0_t[:], wx0.bitcast(F32R))
            w_tiles = {}
            for name, src in (("wh0", wh0), ("wx1", wx1), ("wh1", wh1)):
                wt = wpool.tile([128, KC * H], F32R, tag=name)
                wt_r = wt[:].rearrange("p (k n) -> p k n", k=KC, n=H)
                # per-chunk DMAs so the first matmuls don't wait on 4 MB
                for k in range(KC):
                    nc.sync.dma_start(wt_r[:, k, :], src[k].bitcast(F32R))
                w_tiles[name] = wt_r
            wh0_r, wx1_r, wh1_r = (w_tiles["wh0"], w_tiles["wx1"],
                                   w_tiles["wh1"])

            # x^T resident, columns (t, b) t-major; chunked DMA
            xT_t = xpool.tile([V, t_dev * BL], F32R)
            for xc in range(4):
                w = t_dev * BL // 4
                nc.sync.dma_start(xT_t[:, xc * w:(xc + 1) * w],
                                  xT[:, xc * w:(xc + 1) * w].bitcast(F32R))

            hblk_tiles = []   # rolling h0^T block tiles, one per CH steps
            h1t_tiles = []    # rolling h1^T tiles, one per step
            rep_state = {"rep": 0}

            # ---------- emission helpers ----------
            def b_stat(t, k):
                """stationary h0^T chunk k for layer-0 step t."""
                if t == 0:
                    return hT0_init[:].rearrange(
                        "p (k b) -> p k b", k=KC, b=BL)[:, k, :]
                blk = hblk_tiles[(t - 1) // CH][:].rearrange(
                    "p (k s b) -> p k s b", k=KC, s=CH, b=BL)
                return blk[:, k, (t - 1) % CH, :]

            def emit_b_mms(t, nh):
                """layer-0 step t, half nh: z = [x_t, h_{t-1}] @ [Wx0; Wh0]."""
                if t % CH == 0 and t // CH == len(hblk_tiles):
                    hblk_t = hblk_pool.tile([128, KC * CH * BL], F32R,
                                            tag="hblk")
                    hblk_tiles.append(hblk_t)
                zt = zb_pool.tile([BL, 512], F32, tag="zb")
                psn = zt[:]
                nc.tensor.matmul(psn, xT_t[:, t * BL:(t + 1) * BL],
                                 wx0_t[:, nh * 512:(nh + 1) * 512],
                                 start=True, stop=False)
                if bias0:
                    nc.tensor.matmul(psn, ones_r[:],
                                     b0_t[:, nh * 512:(nh + 1) * 512],
                                     start=False, stop=False)
                for k in range(KC):
                    nc.tensor.matmul(psn, b_stat(t, k),
                                     wh0_r[:, k, nh * 512:(nh + 1) * 512],
                                     start=False, stop=(k == KC - 1))
                if nh == 0:
                    h_rows = hrow0_pool.tile([BL, H], F32, tag="hrow0")
                    emit_b_mms.h_rows = h_rows
                nc.scalar.activation(
                    emit_b_mms.h_rows[:, nh * 512:(nh + 1) * 512], psn, TANH)

            def emit_b_tail(t):
                """layer-0 step t: transpose h rows into the h0^T block."""
                h_rows = emit_b_mms.h_rows
                tp = tps_pool.tile([128, KC * BL], F32, tag="tps")
                for k in range(KC):
                    nc.tensor.transpose(tp[:, k * BL:(k + 1) * BL],
                                        h_rows[:, k * 128:(k + 1) * 128],
                                        eye_f[:])
                blk = hblk_tiles[t // CH][:].rearrange(
                    "p (k s b) -> p k s b", k=KC, s=CH, b=BL)
                nc.scalar.copy(
                    blk[:, :, t % CH, :],
                    tp[:].rearrange("p (k b) -> p k b", k=KC, b=BL))
                if t == t_dev - 1:
                    nc.sync.dma_start(hlast[0], h_rows[:])

            def emit_c_chunk(m):
                """U1 chunk m: rows (t, b) for t in [m*CH, (m+1)*CH)."""
                blk = hblk_tiles[m][:].rearrange(
                    "p (k s b) -> p k s b", k=KC, s=CH, b=BL)
                u_sb = stage_pool.tile([128, H], F32R, tag="stage")
                for nh in range(NH):
                    psn = aps_pool.tile([128, 512], F32, tag="aps")
                    first = True
                    if bias1:
                        nc.tensor.matmul(psn[:], ones_r[:],
                                         b1_t[:, nh * 512:(nh + 1) * 512],
                                         start=True, stop=False)
                        first = False
                    for k in range(KC):
                        nc.tensor.matmul(
                            psn[:],
                            blk[:, k, :, :],
                            wx1_r[:, k, nh * 512:(nh + 1) * 512],
                            start=first, stop=(k == KC - 1))
                        first = False
                    nc.vector.tensor_copy(
                        u_sb[:, nh * 512:(nh + 1) * 512], psn[:])
                nc.sync.dma_start(U1[m * CH:(m + 1) * CH, :, :], u_sb[:])

            def d_stat(t, k):
                if t == 0:
                    return hT1_init[:].rearrange(
                        "p (k b) -> p k b", k=KC, b=BL)[:, k, :]
                return h1t_tiles[t - 1][:].rearrange(
                    "p (k b) -> p k b", k=KC, b=BL)[:, k, :]

            def emit_d_mms(t, nh):
                """layer-1 step t, half nh."""
                if nh == 0:
                    u_t = u1step_pool.tile([BL, H], F32R, tag="u1")
                    nc.sync.dma_start(u_t[:], U1[t, :, :])
                    emit_d_mms.u_t = u_t
                u_t = emit_d_mms.u_t
                zt = zd_pool.tile([BL, 512], F32, tag="zd")
                psn = zt[:]
                for k in range(KC):
                    nc.tensor.matmul(psn, d_stat(t, k),
                                     wh1_r[:, k, nh * 512:(nh + 1) * 512],
                                     start=(k == 0), stop=(k == KC - 1))
                nc.vector.tensor_add(psn, psn,
                                     u_t[:, nh * 512:(nh + 1) * 512])
                if nh == 0:
                    emit_d_mms.h_rows = hrow1_pool.tile([BL, H], F32,
                                                        tag="hrow1")
                nc.scalar.activation(
                    emit_d_mms.h_rows[:, nh * 512:(nh + 1) * 512], psn, TANH)

            def emit_d_tail(t):
                h_rows = emit_d_mms.h_rows
                nc.sync.dma_start(outs[:, t, :], h_rows[:])
                if t == t_dev - 1:
                    nc.sync.dma_start(hlast[1], h_rows[:])
                    return
                tp = tps_pool.tile([128, KC * BL], F32, tag="tps")
                for k in range(KC):
                    nc.tensor.transpose(tp[:, k * BL:(k + 1) * BL],
                                        h_rows[:, k * 128:(k + 1) * 128],
                                        eye_f[:])
                h1t = h1t_pool.tile([128, KC * BL], F32R, tag="h1t")
                nc.scalar.copy(
                    h1t[:].rearrange("p (k b) -> p k b", k=KC, b=BL),
                    tp[:].rearrange("p (k b) -> p k b", k=KC, b=BL))
                h1t_tiles.append(h1t)

            # ---------- wavefront ----------
            # PE order per tick t:
            #   B:MMs(t,0) | D:T(td-1) | B:MMs(t,1) | D:MMs(td,0) | B:T(t)
            #   | D:MMs(td,1) | [C chunk every CH ticks]
            for _rep in range(reps):
                hblk_tiles.clear()
                h1t_tiles.clear()
                for t in range(t_dev + LAG):
                    td = t - LAG
                    if t < t_dev:
                        emit_b_mms(t, 0)
                    if 0 < td < t_dev:
                        emit_d_tail(td - 1)
                    if t < t_dev:
                        emit_b_mms(t, 1)
                    if 0 <= td < t_dev:
                        emit_d_mms(td, 0)
                    if t < t_dev:
                        emit_b_tail(t)
                    if 0 <= td < t_dev:
                        emit_d_mms(td, 1)
                    if t < t_dev and t % CH == CH - 1:
                        emit_c_chunk(t // CH)
                    if td == t_dev - 1:
                        emit_d_tail(td)

    nc.compile()
    return nc


def _prep_inputs(x, h, c, Wx0, Wh0, b0, Wx1, Wh1, b1, t_dev=T):
    x = np.asarray(x, dtype=np.float32)
    h = np.asarray(h, dtype=np.float32)
    shared = {
        "wx0": np.ascontiguousarray(np.asarray(Wx0, np.float32)),
        "wh0": np.ascontiguousarray(
            np.asarray(Wh0, np.float32).reshape(KC, 128, H)),
        "wx1": np.ascontiguousarray(
            np.asarray(Wx1, np.float32).reshape(KC, 128, H)),
        "wh1": np.ascontiguousarray(
            np.asarray(Wh1, np.float32).reshape(KC, 128, H)),
        "b0": np.asarray(b0, np.float32).reshape(1, H),
        "b1": np.asarray(b1, np.float32).reshape(1, H),
        "eye8": np.eye(8, dtype=np.float32),
        "ones1": np.ones((1, 128), dtype=np.float32),
    }
    in_maps = []
    for ci in range(NCORES):
        bs = slice(ci * BL, (ci + 1) * BL)
        x_loc = x[bs, :t_dev]                             # (BL, t, V)
        xT_loc = np.ascontiguousarray(
            x_loc.transpose(2, 1, 0)).reshape(V, t_dev * BL)  # (V, (t b))
        hT = np.empty((2, 128, KC * BL), np.float32)
        for l in range(2):
            hl = h[l, bs].T.reshape(KC, 128, BL)          # (hc, p, b)
            hT[l] = np.ascontiguousarray(
                hl.transpose(1, 0, 2)).reshape(128, KC * BL)
        m = dict(shared)
        m["xT"] = xT_loc
        m["hT_init"] = hT
        in_maps.append(m)
    return in_maps


def _bias_flags(b0, b1):
    return bool(np.any(np.asarray(b0))), bool(np.any(np.asarray(b1)))


def _run(in_maps, t_dev=T, bias0=False, bias1=False, reps=1, trace=False,
         **kw):
    from concourse import bass_utils
    key = (t_dev, bias0, bias1, reps)
    if key not in _cache:
        _cache[key] = _build(t_dev, bias0, bias1, reps)
    nc = _cache[key]
    return bass_utils.run_bass_kernel_spmd(
        nc, in_maps, core_ids=list(range(NCORES)), trace=trace, **kw)


def kernel(x, h, c, Wx0, Wh0, b0, Wx1, Wh1, b1):
    bias0, bias1 = _bias_flags(b0, b1)
    in_maps = _prep_inputs(x, h, c, Wx0, Wh0, b0, Wx1, Wh1, b1)
    res = _run(in_maps, bias0=bias0, bias1=bias1)
    outs = np.concatenate([res.results[ci]["outs"] for ci in range(NCORES)],
                          axis=0)
    h_last = np.concatenate([res.results[ci]["hlast"] for ci in range(NCORES)],
                            axis=1)
    return outs, np.ascontiguousarray(h_last), np.asarray(c, np.float32)


# revision 41
# speedup vs baseline: 22.8532x; 1.0047x over previous
"""Trainium2 Bass kernel for a 2-layer tanh RNN (B=64, T=512, V=128, H=1024).

Data-parallel over batch across 8 NeuronCores (8 rows/core), weights
replicated and SBUF-resident. Per core (v2, software-wavefront):

  - Layer-0 recurrence with x fused in: h0_t = tanh([x_t, h0_{t-1}] @ [Wx0; Wh0] + b0)
    (V=128 is exactly one K-chunk; x^T is SBUF-resident)
  - U1 = H0 @ Wx1 + b1 computed in 16-step chunks from SBUF-resident h0^T
    blocks, staged through DRAM
  - Layer-1 recurrence: h1_t = tanh(U1_t + h1_{t-1} @ Wh1), lagging layer 0
    by LAG steps; its matmuls fill layer-0's serial-chain latency (and vice
    versa) since the PE executes in order.

Recurrent matmul layout: stationary = h^T chunks [K=128, M=8] (tiny
self-load), moving = Wh chunks [K=128, N=512] streamed at full rate in
float32r. U1 is added via a K=8 identity matmul in the same PSUM
accumulation group. h rows -> h^T via PE transpose each step.
"""

import numpy as np

B, T, V, H = 64, 512, 128, 1024
NCORES = 8
BL = B // NCORES  # 8 batch rows per core
KC = H // 128     # 8 contraction chunks per H
NH = H // 512     # 2 free-dim halves
LAG = 16          # layer-1 lag (steps); also the U1 chunk size
CH = 16           # steps per U1 chunk / h0T block

_cache = {}


def _build(t_dev=T, bias0=False, bias1=False, reps=1):
    import concourse.bacc as bacc
    import concourse.tile as tile
    import concourse.mybir as mybir

    F32 = mybir.dt.float32
    F32R = mybir.dt.float32r
    TANH = mybir.ActivationFunctionType.Tanh

    assert t_dev % CH == 0
    n_ch = t_dev // CH

    nc = bacc.Bacc("TRN2", target_bir_lowering=False, debug=False,
                   num_devices=NCORES)

    # ---- I/O ----
    xT = nc.dram_tensor("xT", [V, t_dev * BL], F32, kind="ExternalInput").ap()
    hT_init = nc.dram_tensor("hT_init", [2, 128, KC * BL], F32,
                             kind="ExternalInput").ap()
    wx0 = nc.dram_tensor("wx0", [V, H], F32, kind="ExternalInput").ap()
    wh0 = nc.dram_tensor("wh0", [KC, 128, H], F32, kind="ExternalInput").ap()
    wx1 = nc.dram_tensor("wx1", [KC, 128, H], F32, kind="ExternalInput").ap()
    wh1 = nc.dram_tensor("wh1", [KC, 128, H], F32, kind="ExternalInput").ap()
    b0 = nc.dram_tensor("b0", [1, H], F32, kind="ExternalInput").ap()
    b1 = nc.dram_tensor("b1", [1, H], F32, kind="ExternalInput").ap()
    eye8 = nc.dram_tensor("eye8", [8, 8], F32, kind="ExternalInput").ap()
    ones1 = nc.dram_tensor("ones1", [1, 128], F32, kind="ExternalInput").ap()

    outs = nc.dram_tensor("outs", [BL, t_dev, H], F32,
                          kind="ExternalOutput").ap()
    hlast = nc.dram_tensor("hlast", [2, BL, H], F32,
                           kind="ExternalOutput").ap()

    U1 = nc.dram_tensor("U1", [t_dev, BL, H], F32R).ap()

    with tile.TileContext(nc) as tc:
        with tc.tile_pool(name="consts", bufs=1) as consts, \
             tc.tile_pool(name="wpool", bufs=1) as wpool, \
             tc.tile_pool(name="xpool", bufs=1) as xpool, \
             tc.tile_pool(name="u1step", bufs=4) as u1step_pool, \
             tc.tile_pool(name="hrow0", bufs=2) as hrow0_pool, \
             tc.tile_pool(name="hrow1", bufs=2) as hrow1_pool, \
             tc.tile_pool(name="hblk", bufs=3) as hblk_pool, \
             tc.tile_pool(name="h1t", bufs=2) as h1t_pool, \
             tc.tile_pool(name="stage", bufs=2) as stage_pool, \
             tc.tile_pool(name="zb", bufs=2, space="PSUM") as zb_pool, \
             tc.tile_pool(name="zd", bufs=2, space="PSUM") as zd_pool, \
             tc.tile_pool(name="tps", bufs=2, space="PSUM") as tps_pool, \
             tc.tile_pool(name="aps", bufs=2, space="PSUM") as aps_pool:

            # ---- constants / weights (DMA order = first-use order, so
            # the first matmuls start ~4 us in instead of ~34 us) ----
            xT_t = xpool.tile([V, t_dev * BL], F32R)
            xw = t_dev * BL // 4
            nc.sync.dma_start(xT_t[:, 0:xw], xT[:, 0:xw].bitcast(F32R))
            wx0_t = wpool.tile([V, H], F32R)
            nc.sync.dma_start(wx0_t[:], wx0.bitcast(F32R))

            w_tiles = {}

            def load_w(name, src):
                wt = wpool.tile([128, KC * H], F32R, tag=name)
                wt_r = wt[:].rearrange("p (k n) -> p k n", k=KC, n=H)
                # per-chunk DMAs so the first matmuls don't wait on 4 MB
                for k in range(KC):
                    nc.sync.dma_start(wt_r[:, k, :], src[k].bitcast(F32R))
                w_tiles[name] = wt_r

            load_w("wh0", wh0)
            hT0_init = consts.tile([128, KC * BL], F32R)
            nc.sync.dma_start(hT0_init[:], hT_init[0].bitcast(F32R))
            hT1_init = consts.tile([128, KC * BL], F32R)
            nc.sync.dma_start(hT1_init[:], hT_init[1].bitcast(F32R))
            eye_f = consts.tile([8, 8], F32)
            nc.sync.dma_start(eye_f[:], eye8)
            for xc in range(1, 4):
                nc.sync.dma_start(xT_t[:, xc * xw:(xc + 1) * xw],
                                  xT[:, xc * xw:(xc + 1) * xw].bitcast(F32R))
            load_w("wx1", wx1)
            load_w("wh1", wh1)
            eye_r = consts.tile([8, 8], F32R)
            nc.sync.dma_start(eye_r[:], eye8.bitcast(F32R))
            ones_r = consts.tile([1, 128], F32R)
            nc.sync.dma_start(ones_r[:], ones1.bitcast(F32R))
            b0_t = consts.tile([1, H], F32R)
            nc.sync.dma_start(b0_t[:], b0.bitcast(F32R))
            b1_t = consts.tile([1, H], F32R)
            nc.sync.dma_start(b1_t[:], b1.bitcast(F32R))
            wh0_r, wx1_r, wh1_r = (w_tiles["wh0"], w_tiles["wx1"],
                                   w_tiles["wh1"])

            hblk_tiles = []   # rolling h0^T block tiles, one per CH steps
            h1t_tiles = []    # rolling h1^T tiles, one per step
            rep_state = {"rep": 0}

            # ---------- emission helpers ----------
            def b_stat(t, k):
                """stationary h0^T chunk k for layer-0 step t."""
                if t == 0:
                    return hT0_init[:].rearrange(
                        "p (k b) -> p k b", k=KC, b=BL)[:, k, :]
                blk = hblk_tiles[(t - 1) // CH][:].rearrange(
                    "p (k s b) -> p k s b", k=KC, s=CH, b=BL)
                return blk[:, k, (t - 1) % CH, :]

            def emit_b_mms(t, nh):
                """layer-0 step t, half nh: z = [x_t, h_{t-1}] @ [Wx0; Wh0]."""
                if t % CH == 0 and t // CH == len(hblk_tiles):
                    hblk_t = hblk_pool.tile([128, KC * CH * BL], F32R,
                                            tag="hblk")
                    hblk_tiles.append(hblk_t)
                zt = zb_pool.tile([BL, 512], F32, tag="zb")
                psn = zt[:]
                nc.tensor.matmul(psn, xT_t[:, t * BL:(t + 1) * BL],
                                 wx0_t[:, nh * 512:(nh + 1) * 512],
                                 start=True, stop=False)
                if bias0:
                    nc.tensor.matmul(psn, ones_r[:],
                                     b0_t[:, nh * 512:(nh + 1) * 512],
                                     start=False, stop=False)
                for k in range(KC):
                    nc.tensor.matmul(psn, b_stat(t, k),
                                     wh0_r[:, k, nh * 512:(nh + 1) * 512],
                                     start=False, stop=(k == KC - 1))
                if nh == 0:
                    h_rows = hrow0_pool.tile([BL, H], F32, tag="hrow0")
                    emit_b_mms.h_rows = h_rows
                nc.scalar.activation(
                    emit_b_mms.h_rows[:, nh * 512:(nh + 1) * 512], psn, TANH)

            def emit_b_tail(t):
                """layer-0 step t: transpose h rows into the h0^T block."""
                h_rows = emit_b_mms.h_rows
                tp = tps_pool.tile([128, KC * BL], F32, tag="tps")
                for k in range(KC):
                    nc.tensor.transpose(tp[:, k * BL:(k + 1) * BL],
                                        h_rows[:, k * 128:(k + 1) * 128],
                                        eye_f[:])
                blk = hblk_tiles[t // CH][:].rearrange(
                    "p (k s b) -> p k s b", k=KC, s=CH, b=BL)
                nc.scalar.copy(
                    blk[:, :, t % CH, :],
                    tp[:].rearrange("p (k b) -> p k b", k=KC, b=BL))
                if t == t_dev - 1:
                    nc.sync.dma_start(hlast[0], h_rows[:])

            def emit_c_chunk(m):
                """U1 chunk m: rows (t, b) for t in [m*CH, (m+1)*CH)."""
                blk = hblk_tiles[m][:].rearrange(
                    "p (k s b) -> p k s b", k=KC, s=CH, b=BL)
                u_sb = stage_pool.tile([128, H], F32R, tag="stage")
                for nh in range(NH):
                    psn = aps_pool.tile([128, 512], F32, tag="aps")
                    first = True
                    if bias1:
                        nc.tensor.matmul(psn[:], ones_r[:],
                                         b1_t[:, nh * 512:(nh + 1) * 512],
                                         start=True, stop=False)
                        first = False
                    for k in range(KC):
                        nc.tensor.matmul(
                            psn[:],
                            blk[:, k, :, :],
                            wx1_r[:, k, nh * 512:(nh + 1) * 512],
                            start=first, stop=(k == KC - 1))
                        first = False
                    nc.vector.tensor_copy(
                        u_sb[:, nh * 512:(nh + 1) * 512], psn[:])
                nc.sync.dma_start(U1[m * CH:(m + 1) * CH, :, :], u_sb[:])

            def d_stat(t, k):
                if t == 0:
                    return hT1_init[:].rearrange(
                        "p (k b) -> p k b", k=KC, b=BL)[:, k, :]
                return h1t_tiles[t - 1][:].rearrange(
                    "p (k b) -> p k b", k=KC, b=BL)[:, k, :]

            def emit_d_mms(t, nh):
                """layer-1 step t, half nh."""
                if nh == 0:
                    u_t = u1step_pool.tile([BL, H], F32R, tag="u1")
                    nc.sync.dma_start(u_t[:], U1[t, :, :])
                    emit_d_mms.u_t = u_t
                u_t = emit_d_mms.u_t
                zt = zd_pool.tile([BL, 512], F32, tag="zd")
                psn = zt[:]
                for k in range(KC):
                    nc.tensor.matmul(psn, d_stat(t, k),
                                     wh1_r[:, k, nh * 512:(nh + 1) * 512],
                                     start=(k == 0), stop=(k == KC - 1))
                nc.vector.tensor_add(psn, psn,
                                     u_t[:, nh * 512:(nh + 1) * 512])
                if nh == 0:
                    emit_d_mms.h_rows = hrow1_pool.tile([BL, H], F32,
                                                        tag="hrow1")
                nc.scalar.activation(
                    emit_d_mms.h_rows[:, nh * 512:(nh + 1) * 512], psn, TANH)

            def emit_d_tail(t):
                h_rows = emit_d_mms.h_rows
                nc.sync.dma_start(outs[:, t, :], h_rows[:])
                if t == t_dev - 1:
                    nc.sync.dma_start(hlast[1], h_rows[:])
                    return
                tp = tps_pool.tile([128, KC * BL], F32, tag="tps")
                for k in range(KC):
                    nc.tensor.transpose(tp[:, k * BL:(k + 1) * BL],
                                        h_rows[:, k * 128:(k + 1) * 128],
                                        eye_f[:])
                h1t = h1t_pool.tile([128, KC * BL], F32R, tag="h1t")
                nc.scalar.copy(
                    h1t[:].rearrange("p (k b) -> p k b", k=KC, b=BL),
                    tp[:].rearrange("p (k b) -> p k b", k=KC, b=BL))
                h1t_tiles.append(h1t)

            # ---------- wavefront ----------
            # PE order per tick t:
            #   B:MMs(t,0) | D:T(td-1) | B:MMs(t,1) | D:MMs(td,0) | B:T(t)
            #   | D:MMs(td,1) | [C chunk every CH ticks]
            for _rep in range(reps):
                hblk_tiles.clear()
                h1t_tiles.clear()
                for t in range(t_dev + LAG):
                    td = t - LAG
                    if t < t_dev:
                        emit_b_mms(t, 0)
                    if 0 < td < t_dev:
                        emit_d_tail(td - 1)
                    if t < t_dev:
                        emit_b_mms(t, 1)
                    if 0 <= td < t_dev:
                        emit_d_mms(td, 0)
                    if t < t_dev:
                        emit_b_tail(t)
                    if 0 <= td < t_dev:
                        emit_d_mms(td, 1)
                    if t < t_dev and t % CH == CH - 1:
                        emit_c_chunk(t // CH)
                    if td == t_dev - 1:
                        emit_d_tail(td)

    nc.compile()
    return nc


def _prep_inputs(x, h, c, Wx0, Wh0, b0, Wx1, Wh1, b1, t_dev=T):
    x = np.asarray(x, dtype=np.float32)
    h = np.asarray(h, dtype=np.float32)
    shared = {
        "wx0": np.ascontiguousarray(np.asarray(Wx0, np.float32)),
        "wh0": np.ascontiguousarray(
            np.asarray(Wh0, np.float32).reshape(KC, 128, H)),
        "wx1": np.ascontiguousarray(
            np.asarray(Wx1, np.float32).reshape(KC, 128, H)),
        "wh1": np.ascontiguousarray(
            np.asarray(Wh1, np.float32).reshape(KC, 128, H)),
        "b0": np.asarray(b0, np.float32).reshape(1, H),
        "b1": np.asarray(b1, np.float32).reshape(1, H),
        "eye8": np.eye(8, dtype=np.float32),
        "ones1": np.ones((1, 128), dtype=np.float32),
    }
    in_maps = []
    for ci in range(NCORES):
        bs = slice(ci * BL, (ci + 1) * BL)
        x_loc = x[bs, :t_dev]                             # (BL, t, V)
        xT_loc = np.ascontiguousarray(
            x_loc.transpose(2, 1, 0)).reshape(V, t_dev * BL)  # (V, (t b))
        hT = np.empty((2, 128, KC * BL), np.float32)
        for l in range(2):
            hl = h[l, bs].T.reshape(KC, 128, BL)          # (hc, p, b)
            hT[l] = np.ascontiguousarray(
                hl.transpose(1, 0, 2)).reshape(128, KC * BL)
        m = dict(shared)
        m["xT"] = xT_loc
        m["hT_init"] = hT
        in_maps.append(m)
    return in_maps


def _bias_flags(b0, b1):
    return bool(np.any(np.asarray(b0))), bool(np.any(np.asarray(b1)))


def _run(in_maps, t_dev=T, bias0=False, bias1=False, reps=1, trace=False,
         **kw):
    from concourse import bass_utils
    key = (t_dev, bias0, bias1, reps)
    if key not in _cache:
        _cache[key] = _build(t_dev, bias0, bias1, reps)
    nc = _cache[key]
    return bass_utils.run_bass_kernel_spmd(
        nc, in_maps, core_ids=list(range(NCORES)), trace=trace, **kw)


def kernel(x, h, c, Wx0, Wh0, b0, Wx1, Wh1, b1):
    bias0, bias1 = _bias_flags(b0, b1)
    in_maps = _prep_inputs(x, h, c, Wx0, Wh0, b0, Wx1, Wh1, b1)
    res = _run(in_maps, bias0=bias0, bias1=bias1)
    outs = np.concatenate([res.results[ci]["outs"] for ci in range(NCORES)],
                          axis=0)
    h_last = np.concatenate([res.results[ci]["hlast"] for ci in range(NCORES)],
                            axis=1)
    return outs, np.ascontiguousarray(h_last), np.asarray(c, np.float32)
